# revision 36
# baseline (speedup 1.0000x reference)
"""GQA attention layer (dense_transformer) on 8 Trainium2 NeuronCores.

Tensor-parallel over heads: each core gets 4 q-heads + 1 kv-head (shard of
wq/wk/wv output dims and wo input dim), hidden_states replicated; partial
o_proj outputs are summed on the host (the all-reduce).

Per-core pipeline (all matmuls bf16 with fp32 PSUM accumulation):
  phase 1: qkv projections from host-pretransposed hsT tiles; fused RMSNorm
    (norm_w folded into host-precomputed RoPE tables) + RoPE; PE-transpose
    q/k into [d, t]. Transposes for tile i are emitted after tile i+1's
    matmuls so the PE never waits on the vector-engine norm chain.
  phase 2 (per batch, per 512-wide sq block j, per head):
    scoresT[sk,sq] = k_tile @ qT, two k-tiles paired into one 2-bank PSUM
    tile so a single ACT exp covers both (halves ACT instruction count);
    causal mask via 0/1 multiply on diagonal straddlers; PV with STATIONARY
    V: opvT[d,sq] += V_k^T @ probs_k -- one 512-wide matmul per k-tile whose
    output lands pre-transposed for o_proj (no output transposes).
    Softmax denominator: probs pair-summed on DVE (bf16), folded, one
    ones-vector matmul -> denom row, reciprocal_approx_fast (DVE),
    partition_broadcast (Pool), normalize fused into the opvT PSUM->SBUF
    copy. o_proj matmul groups for block j-1 are interleaved into the
    k-loops as always-ready PE filler so the tensor engine never idles on
    exp; their PSUM->SBUF staging copies alternate ACT/Pool.
"""

import numpy as np
import ml_dtypes

H, KV, D, HID = 32, 8, 128, 4096
B, S = 2, 2048
T = B * S
NCORES = 8
HL = H // NCORES          # 4 q heads per core
QF = HL * D               # 512
EPS = 1e-6
THETA = 10000.0
SCALE = 1.0 / float(np.sqrt(D))

_NC_CACHE = {}


def _build():
    import concourse.bacc as bacc
    import concourse.mybir as mybir
    import concourse.tile as tile
    from concourse.masks import make_identity

    fp32 = mybir.dt.float32
    bf16 = mybir.dt.bfloat16

    nc = bacc.Bacc("TRN2", target_bir_lowering=False)

    NT = T // 128            # 32 token tiles
    NTB = S // 128           # 16 token tiles per batch
    NC = HID // 128          # 32 contraction chunks
    NJ = 4                   # 512-wide sq blocks per batch
    NO = HID // 512          # o_proj output chunks

    # all inputs host-pretransposed to partition-major layouts so every DMA
    # line is contiguous per partition (128 big descriptors, not 4096 tiny)
    hsT = nc.dram_tensor("hsT", [128, NT, NC, 128], bf16, kind="ExternalInput")
    wq = nc.dram_tensor("wq", [128, NC, QF], bf16, kind="ExternalInput")
    wkv = nc.dram_tensor("wkv", [128, NC, 2 * D], bf16, kind="ExternalInput")
    wo = nc.dram_tensor("wo", [128, HL, HID], bf16, kind="ExternalInput")
    cosq = nc.dram_tensor("cosq", [128, NTB, D], bf16, kind="ExternalInput")
    sinq = nc.dram_tensor("sinq", [128, NTB, D], bf16, kind="ExternalInput")
    cosk = nc.dram_tensor("cosk", [128, NTB, D], bf16, kind="ExternalInput")
    sink = nc.dram_tensor("sink", [128, NTB, D], bf16, kind="ExternalInput")
    out = nc.dram_tensor("out", [T, HID], bf16, kind="ExternalOutput")

    with tile.TileContext(nc) as tc:
        with (
            tc.tile_pool(name="persist", bufs=1) as persist,
            tc.tile_pool(name="hst", bufs=3) as hstp,
            tc.tile_pool(name="work", bufs=3) as work,
            tc.tile_pool(name="probs", bufs=3) as probsp,
            tc.tile_pool(name="stats", bufs=8) as stats,
            tc.tile_pool(name="otb", bufs=3) as otbp,
            tc.tile_pool(name="prsp", bufs=2) as prsp,
            tc.tile_pool(name="rbcp", bufs=1) as rbcp,
            tc.tile_pool(name="ostage", bufs=3) as ostage,
        ):
            # ---- persistent constants / weights ----
            ident = persist.tile([128, 128], bf16)
            make_identity(nc, ident)
            eps_t = persist.tile([128, 1], fp32)
            nc.vector.memset(eps_t, EPS)
            ones_col = persist.tile([128, 1], bf16)
            nc.vector.memset(ones_col, 1.0)

            # one master causal pattern; mask for straddle index i is the
            # [.., (3-i)*128 : (3-i)*128+512] slice (keep where col >= p+128i)
            masks_m = persist.tile([128, 896], bf16)
            nc.gpsimd.memset(masks_m, 1.0)
            nc.gpsimd.affine_select(
                out=masks_m, in_=masks_m,
                compare_op=mybir.AluOpType.is_ge,
                fill=0.0, base=-384,
                pattern=[[1, 896]], channel_multiplier=-1,
            )

            def mask_slice(i):
                return masks_m[:, (3 - i) * 128:(3 - i) * 128 + 512]

            # chunked weight loads so the first matmuls start early; the
            # first hst tile (as two half-chunk tiles) is requested before
            # any weight chunk
            HC = NC // 2
            hsT_h = hsT.rearrange("p i (u c) t -> p i u c t", u=2)
            hst_early = [hstp.tile([128, HC, 128], bf16, name=f"hste{u}",
                                   tag="h")
                         for u in range(2)]
            for u in range(2):
                nc.sync.dma_start(out=hst_early[u], in_=hsT_h[:, 0, u, :, :])
            wq_sb = persist.tile([128, NC, QF], bf16)
            wkv_sb = persist.tile([128, NC, 2 * D], bf16)
            for c0 in range(0, NC, 8):
                nc.sync.dma_start(out=wq_sb[:, c0:c0 + 8, :],
                                  in_=wq[:, c0:c0 + 8, :])
                nc.sync.dma_start(out=wkv_sb[:, c0:c0 + 8, :],
                                  in_=wkv[:, c0:c0 + 8, :])
            wo_sb = persist.tile([128, HL, HID], bf16)
            nc.sync.dma_start(out=wo_sb, in_=wo[:, :, :])

            tabs = {}
            for name, t in (("cosq", cosq), ("sinq", sinq), ("cosk", cosk), ("sink", sink)):
                tt = persist.tile([128, NTB, D], bf16, name=f"tab_{name}")
                nc.sync.dma_start(out=tt, in_=t[:, :, :])
                tabs[name] = tt

            # ---- persistent activations ----
            QT = [persist.tile([128, T], bf16, name=f"QT{h}") for h in range(HL)]
            KT = persist.tile([128, T], bf16)                       # [d, t]
            VA = persist.tile([128, NT, D], bf16)                   # [sk, d] per tile

            # ================= phase 1: projections + norm + rope =================
            with (
                tc.tile_pool(name="psQ", bufs=3, space="PSUM") as psQ,
                tc.tile_pool(name="psKV", bufs=3, space="PSUM") as psKV,
                tc.tile_pool(name="psT", bufs=2, space="PSUM") as psT,
            ):
                def norm_rope_transpose(psum_slice, cos_t, sin_t, dstT, tcol):
                    ssq = stats.tile([128, 1], fp32, tag="ssq")
                    scratch = work.tile([128, 128], bf16, tag="sq")
                    nc.scalar.activation(
                        out=scratch, in_=psum_slice,
                        func=mybir.ActivationFunctionType.Square,
                        accum_out=ssq,
                    )
                    rstd = stats.tile([128, 1], fp32, tag="rstd")
                    nc.scalar.activation(
                        out=rstd, in_=ssq, func=mybir.ActivationFunctionType.Sqrt,
                        bias=eps_t, scale=1.0 / D,
                    )
                    nc.vector.reciprocal(out=rstd, in_=rstd)

                    ynorm = work.tile([128, 128], bf16, tag="ynorm")
                    shifted = work.tile([128, 128], bf16, tag="shifted")
                    nc.vector.tensor_scalar_mul(out=ynorm, in0=psum_slice, scalar1=rstd)
                    nc.vector.tensor_scalar_mul(
                        out=shifted[:, 0:64], in0=psum_slice[:, 64:128], scalar1=rstd)
                    nc.vector.tensor_scalar_mul(
                        out=shifted[:, 64:128], in0=psum_slice[:, 0:64], scalar1=rstd)
                    rot = work.tile([128, 128], bf16, tag="rot")
                    nc.vector.tensor_mul(out=rot, in0=ynorm, in1=cos_t)
                    nc.vector.tensor_mul(out=shifted, in0=shifted, in1=sin_t)
                    nc.vector.tensor_add(out=rot, in0=rot, in1=shifted)

                    ptr = psT.tile([128, 128], bf16, tag="tr")
                    nc.tensor.transpose(ptr, rot, ident)
                    nc.any.tensor_copy(out=dstT[:, tcol:tcol + 128], in_=ptr)

                def finish_tile(pq, pkv, i):
                    si = i % NTB
                    for h in range(HL):
                        norm_rope_transpose(
                            pq[:, h * D:(h + 1) * D],
                            tabs["cosq"][:, si, :], tabs["sinq"][:, si, :],
                            QT[h], i * 128)
                    norm_rope_transpose(
                        pkv[:, 0:D],
                        tabs["cosk"][:, si, :], tabs["sink"][:, si, :],
                        KT, i * 128)
                    nc.any.tensor_copy(out=VA[:, i, :], in_=pkv[:, D:2 * D])

                pend = None
                for i in range(NT):
                    if i == 0:
                        halves = hst_early
                    else:
                        halves = []
                        for u in range(2):
                            hh = hstp.tile([128, HC, 128], bf16, name="hsth",
                                           tag="h")
                            nc.sync.dma_start(out=hh, in_=hsT_h[:, i, u, :, :])
                            halves.append(hh)
                    pq = psQ.tile([128, QF], fp32, tag="Q")
                    pkv = psKV.tile([128, 2 * D], fp32, tag="KV")
                    for c in range(NC):
                        hst_c = halves[c // HC][:, c % HC, :]
                        nc.tensor.matmul(pq, hst_c, wq_sb[:, c, :],
                                         start=(c == 0), stop=(c == NC - 1))
                        nc.tensor.matmul(pkv, hst_c, wkv_sb[:, c, :],
                                         start=(c == 0), stop=(c == NC - 1))
                    if pend is not None:
                        finish_tile(*pend)
                    pend = (pq, pkv, i)
                finish_tile(*pend)

            # ============ phase 2: attention with interleaved o_proj ============
            with (
                tc.tile_pool(name="psS", bufs=2, space="PSUM") as psS,
                tc.tile_pool(name="psV", bufs=2, space="PSUM") as psV,
                tc.tile_pool(name="psO", bufs=2, space="PSUM") as psO,
            ):
                fill = []      # pending o_proj emitters (always-ready PE work)
                copy_flip = [0]

                def drain_fill(n):
                    for _ in range(min(n, len(fill))):
                        fill.pop(0)()

                def make_oproj_group(ot_blk, b, j, it, n):
                    def emit():
                        po = psO.tile([128, 512], fp32, tag="O")
                        for h in range(HL):
                            nc.tensor.matmul(
                                po,
                                ot_blk[:, h, it * 128:(it + 1) * 128],
                                wo_sb[:, h, n * 512:(n + 1) * 512],
                                start=(h == 0), stop=(h == HL - 1))
                        ost = ostage.tile([128, 512], bf16, tag="ost")
                        copy_flip[0] ^= 1
                        if copy_flip[0]:
                            nc.scalar.copy(out=ost, in_=po)
                        else:
                            nc.vector.tensor_copy(out=ost, in_=po)
                        t0 = b * S + j * 512 + it * 128
                        nc.sync.dma_start(
                            out=out[t0:t0 + 128, n * 512:(n + 1) * 512], in_=ost)
                    return emit

                for b in range(B):
                    t0 = b * S
                    k0 = b * NTB
                    for j in range(NJ):
                        qcol = t0 + j * 512
                        K = 4 * (j + 1)
                        NP = K // 2
                        ot_blk = otbp.tile([128, HL, 512], bf16, name="otb")
                        for h in range(HL):
                            prs = prsp.tile([128, 2, 512], bf16, tag="prsum")
                            opvT = psV.tile([128, 512], fp32, tag="V")
                            # pair 0's exp writes straight into the prs
                            # accumulator; later pairs are added into it only
                            # AFTER their (lagged) PV matmuls have consumed
                            # them, so in-place accumulation never races PV.
                            pvq = []   # pending (pair_idx, prpair) lagged PV+add
                            def flush_pv(pp, prpair):
                                for half in range(2):
                                    kk = 2 * pp + half
                                    nc.tensor.matmul(
                                        opvT, VA[:, k0 + kk, :],
                                        prpair[:, half, :],
                                        start=(kk == 0), stop=(kk == K - 1))
                                if pp > 0:
                                    nc.vector.tensor_add(
                                        out=prs, in0=prs, in1=prpair)
                            for p in range(NP):
                                spair = psS.tile([128, 2, 512], fp32, tag="S")
                                for half in range(2):
                                    k = 2 * p + half
                                    nc.tensor.matmul(
                                        spair[:, half, :],
                                        KT[:, t0 + k * 128: t0 + (k + 1) * 128],
                                        QT[h][:, qcol:qcol + 512],
                                        start=True, stop=True)
                                if p == 0:
                                    prpair = prs
                                else:
                                    prpair = probsp.tile(
                                        [128, 2, 512], bf16, tag="pr")
                                nc.scalar.activation(
                                    out=prpair, in_=spair,
                                    func=mybir.ActivationFunctionType.Exp,
                                    scale=SCALE)
                                for half in range(2):
                                    k = 2 * p + half
                                    if k >= 4 * j:
                                        nc.gpsimd.tensor_mul(
                                            out=prpair[:, half, :],
                                            in0=prpair[:, half, :],
                                            in1=mask_slice(k - 4 * j))
                                pvq.append((p, prpair))
                                drain_fill(2)
                                if len(pvq) > 1:
                                    flush_pv(*pvq.pop(0))
                            for pp, prpair in pvq:
                                flush_pv(pp, prpair)
                            # denominator: fold pair halves in place, ones^T @
                            # prs[:,0] -> [1,512], fast reciprocal, broadcast,
                            # normalize during the opvT PSUM->SBUF copy.
                            nc.vector.tensor_add(
                                out=prs[:, 0, :], in0=prs[:, 0, :],
                                in1=prs[:, 1, :])
                            dn = psV.tile([1, 512], fp32, tag="V")
                            nc.tensor.matmul(dn, ones_col, prs[:, 0, :],
                                             start=True, stop=True)
                            rbc = rbcp.tile([128, 512], fp32, tag="rbc")
                            nc.vector.reciprocal_approx_fast(
                                out=rbc[0:1, :], in_=dn)
                            nc.gpsimd.partition_broadcast(rbc, rbc[0:1, :])
                            nc.vector.tensor_mul(
                                out=ot_blk[:, h, :], in0=opvT, in1=rbc)
                        fill.extend(
                            make_oproj_group(ot_blk, b, j, it, n)
                            for it in range(4) for n in range(NO))
                drain_fill(len(fill))

    nc.finalize()
    return nc


def _get_nc():
    if "nc" not in _NC_CACHE:
        _NC_CACHE["nc"] = _build()
    return _NC_CACHE["nc"]


def _host_prep(hidden_states, wq, wk, wv, wo, q_norm_w, k_norm_w, position_ids):
    bf = ml_dtypes.bfloat16
    NT, NTB, NC_, HL_ = T // 128, S // 128, HID // 128, HL
    hs = np.asarray(hidden_states, dtype=np.float32).reshape(T, HID)
    # [128(p), NT, NC, 128(t)]: partition-major so DMA lines are contiguous
    hsT = np.ascontiguousarray(
        hs.reshape(NT, 128, NC_, 128).transpose(3, 0, 2, 1)).astype(bf)

    # RoPE tables with norm weights folded in (positions are identical
    # across batches for this problem's arange position_ids).
    pos = np.asarray(position_ids)[0].astype(np.float64)
    inv_freq = 1.0 / (THETA ** (np.arange(0, D, 2, dtype=np.float64) / D))
    ang = pos[:, None] * inv_freq
    emb = np.concatenate([ang, ang], axis=-1)
    cos = np.cos(emb).astype(np.float32)
    sin = np.sin(emb).astype(np.float32)

    def fold(w):
        w = np.asarray(w, dtype=np.float32)
        w_shift = np.concatenate([w[D // 2:], w[:D // 2]])
        sgn = np.concatenate([-np.ones(D // 2, np.float32), np.ones(D // 2, np.float32)])
        return (cos * w).astype(bf), (sin * w_shift * sgn).astype(bf)

    def ptile(a, nblk):
        # [nblk*128, F] -> [128, nblk, F] partition-major
        a = np.ascontiguousarray(a)
        return np.ascontiguousarray(
            a.reshape(nblk, 128, a.shape[-1]).transpose(1, 0, 2))

    cq, sq_ = fold(q_norm_w)
    ck, sk_ = fold(k_norm_w)
    cq, sq_, ck, sk_ = (ptile(x, NTB) for x in (cq, sq_, ck, sk_))

    wq = np.asarray(wq, dtype=np.float32)
    wk = np.asarray(wk, dtype=np.float32)
    wv = np.asarray(wv, dtype=np.float32)
    wo = np.asarray(wo, dtype=np.float32)

    in_maps = []
    for c in range(NCORES):
        qs = slice(c * QF, (c + 1) * QF)
        ks = slice(c * D, (c + 1) * D)
        in_maps.append({
            "hsT": hsT,
            "wq": ptile(wq[:, qs].astype(bf), NC_),
            "wkv": ptile(np.concatenate(
                [wk[:, ks], wv[:, ks]], axis=1).astype(bf), NC_),
            "wo": ptile(wo[qs, :].astype(bf), HL_),
            "cosq": cq, "sinq": sq_, "cosk": ck, "sink": sk_,
        })
    return in_maps


def kernel(hidden_states, wq, wk, wv, wo, q_norm_w, k_norm_w, position_ids,
           _trace=False):
    from concourse.bass_utils import run_bass_kernel_spmd

    nc = _get_nc()
    in_maps = _host_prep(hidden_states, wq, wk, wv, wo,
                         q_norm_w, k_norm_w, position_ids)
    res = run_bass_kernel_spmd(nc, in_maps, core_ids=list(range(NCORES)),
                               trace=_trace)
    total = np.zeros((T, HID), dtype=np.float32)
    for r in res.results:
        total += r["out"]
    out = total.reshape(B, S, HID)
    if _trace:
        return out, res
    return out


# revision 37
# speedup vs baseline: 1.5096x; 1.5096x over previous
"""GQA attention layer (dense_transformer) on 8 Trainium2 NeuronCores.

Tensor-parallel over heads: each core gets 4 q-heads + 1 kv-head (shard of
wq/wk/wv output dims and wo input dim), hidden_states replicated; partial
o_proj outputs are summed on the host (the all-reduce).

Per-core pipeline (all matmuls bf16 with fp32 PSUM accumulation):
  phase 1: qkv projections from host-pretransposed hsT tiles; fused RMSNorm
    (norm_w folded into host-precomputed RoPE tables) + RoPE; PE-transpose
    q/k into [d, t]. Transposes for tile i are emitted after tile i+1's
    matmuls so the PE never waits on the vector-engine norm chain.
  phase 2 (per batch, per 512-wide sq block j, per head):
    scoresT[sk,sq] = k_tile @ qT, two k-tiles paired into one 2-bank PSUM
    tile so a single ACT exp covers both (halves ACT instruction count);
    causal mask via 0/1 multiply on diagonal straddlers; PV with STATIONARY
    V: opvT[d,sq] += V_k^T @ probs_k -- one 512-wide matmul per k-tile whose
    output lands pre-transposed for o_proj (no output transposes).
    Softmax denominator: probs pair-summed on DVE (bf16), folded, one
    ones-vector matmul -> denom row, reciprocal_approx_fast (DVE),
    partition_broadcast (Pool), normalize fused into the opvT PSUM->SBUF
    copy. o_proj matmul groups for block j-1 are interleaved into the
    k-loops as always-ready PE filler so the tensor engine never idles on
    exp; their PSUM->SBUF staging copies alternate ACT/Pool.
"""

import numpy as np
import ml_dtypes

H, KV, D, HID = 32, 8, 128, 4096
B, S = 2, 2048
T = B * S
NCORES = 8
HL = H // NCORES          # 4 q heads per core
QF = HL * D               # 512
EPS = 1e-6
THETA = 10000.0
SCALE = 1.0 / float(np.sqrt(D))

_NC_CACHE = {}


def _build():
    import concourse.bacc as bacc
    import concourse.mybir as mybir
    import concourse.tile as tile
    from concourse.masks import make_identity

    fp32 = mybir.dt.float32
    bf16 = mybir.dt.bfloat16

    nc = bacc.Bacc("TRN2", target_bir_lowering=False)

    NT = T // 128            # 32 token tiles
    NTB = S // 128           # 16 token tiles per batch
    NC = HID // 128          # 32 contraction chunks
    NJ = 4                   # 512-wide sq blocks per batch
    NO = HID // 512          # o_proj output chunks

    # all inputs host-pretransposed to partition-major layouts so every DMA
    # line is contiguous per partition (128 big descriptors, not 4096 tiny)
    hsT = nc.dram_tensor("hsT", [128, NT, NC, 128], bf16, kind="ExternalInput")
    wq = nc.dram_tensor("wq", [128, NC, QF], bf16, kind="ExternalInput")
    wkv = nc.dram_tensor("wkv", [128, NC, 2 * D], bf16, kind="ExternalInput")
    wo = nc.dram_tensor("wo", [128, HL, HID], bf16, kind="ExternalInput")
    cosq = nc.dram_tensor("cosq", [128, NTB, D], bf16, kind="ExternalInput")
    sinq = nc.dram_tensor("sinq", [128, NTB, D], bf16, kind="ExternalInput")
    cosk = nc.dram_tensor("cosk", [128, NTB, D], bf16, kind="ExternalInput")
    sink = nc.dram_tensor("sink", [128, NTB, D], bf16, kind="ExternalInput")
    out = nc.dram_tensor("out", [T, HID], bf16, kind="ExternalOutput")

    with tile.TileContext(nc) as tc:
        with (
            tc.tile_pool(name="persist", bufs=1) as persist,
            tc.tile_pool(name="hst", bufs=3) as hstp,
            tc.tile_pool(name="work", bufs=3) as work,
            tc.tile_pool(name="probs", bufs=3) as probsp,
            tc.tile_pool(name="stats", bufs=8) as stats,
            tc.tile_pool(name="otb", bufs=3) as otbp,
            tc.tile_pool(name="prsp", bufs=2) as prsp,
            tc.tile_pool(name="rbcp", bufs=1) as rbcp,
            tc.tile_pool(name="ostage", bufs=3) as ostage,
        ):
            # ---- persistent constants / weights ----
            ident = persist.tile([128, 128], bf16)
            make_identity(nc, ident)
            eps_t = persist.tile([128, 1], fp32)
            nc.vector.memset(eps_t, EPS)
            ones_col = persist.tile([128, 1], bf16)
            nc.vector.memset(ones_col, 1.0)

            # one master causal pattern; mask for straddle index i is the
            # [.., (3-i)*128 : (3-i)*128+512] slice (keep where col >= p+128i)
            masks_m = persist.tile([128, 896], bf16)
            nc.gpsimd.memset(masks_m, 1.0)
            nc.gpsimd.affine_select(
                out=masks_m, in_=masks_m,
                compare_op=mybir.AluOpType.is_ge,
                fill=0.0, base=-384,
                pattern=[[1, 896]], channel_multiplier=-1,
            )

            def mask_slice(i):
                return masks_m[:, (3 - i) * 128:(3 - i) * 128 + 512]

            # chunked weight loads so the first matmuls start early; the
            # first hst tile (as two half-chunk tiles) is requested before
            # any weight chunk
            HC = NC // 2
            hsT_h = hsT.rearrange("p i (u c) t -> p i u c t", u=2)
            hst_early = [hstp.tile([128, HC, 128], bf16, name=f"hste{u}",
                                   tag="h")
                         for u in range(2)]
            for u in range(2):
                nc.sync.dma_start(out=hst_early[u], in_=hsT_h[:, 0, u, :, :])
            wq_sb = persist.tile([128, NC, QF], bf16)
            wkv_sb = persist.tile([128, NC, 2 * D], bf16)
            for c0 in range(0, NC, 8):
                nc.sync.dma_start(out=wq_sb[:, c0:c0 + 8, :],
                                  in_=wq[:, c0:c0 + 8, :])
                nc.sync.dma_start(out=wkv_sb[:, c0:c0 + 8, :],
                                  in_=wkv[:, c0:c0 + 8, :])
            wo_sb = persist.tile([128, HL, HID], bf16)
            nc.sync.dma_start(out=wo_sb, in_=wo[:, :, :])

            tabs = {}
            for name, t in (("cosq", cosq), ("sinq", sinq), ("cosk", cosk), ("sink", sink)):
                tt = persist.tile([128, NTB, D], bf16, name=f"tab_{name}")
                nc.sync.dma_start(out=tt, in_=t[:, :, :])
                tabs[name] = tt

            # ---- persistent activations ----
            QT = [persist.tile([128, T], bf16, name=f"QT{h}") for h in range(HL)]
            KT = persist.tile([128, T], bf16)                       # [d, t]
            VA = persist.tile([128, NT, D], bf16)                   # [sk, d] per tile

            # ================= phase 1: projections + norm + rope =================
            with (
                tc.tile_pool(name="psQ", bufs=3, space="PSUM") as psQ,
                tc.tile_pool(name="psKV", bufs=3, space="PSUM") as psKV,
                tc.tile_pool(name="psT", bufs=2, space="PSUM") as psT,
            ):
                def norm_rope_transpose(psum_slice, cos_t, sin_t, dstT, tcol):
                    ssq = stats.tile([128, 1], fp32, tag="ssq")
                    scratch = work.tile([128, 128], bf16, tag="sq")
                    nc.scalar.activation(
                        out=scratch, in_=psum_slice,
                        func=mybir.ActivationFunctionType.Square,
                        accum_out=ssq,
                    )
                    rstd = stats.tile([128, 1], fp32, tag="rstd")
                    nc.scalar.activation(
                        out=rstd, in_=ssq, func=mybir.ActivationFunctionType.Sqrt,
                        bias=eps_t, scale=1.0 / D,
                    )
                    nc.vector.reciprocal(out=rstd, in_=rstd)

                    ynorm = work.tile([128, 128], bf16, tag="ynorm")
                    shifted = work.tile([128, 128], bf16, tag="shifted")
                    nc.vector.tensor_scalar_mul(out=ynorm, in0=psum_slice, scalar1=rstd)
                    nc.vector.tensor_scalar_mul(
                        out=shifted[:, 0:64], in0=psum_slice[:, 64:128], scalar1=rstd)
                    nc.vector.tensor_scalar_mul(
                        out=shifted[:, 64:128], in0=psum_slice[:, 0:64], scalar1=rstd)
                    rot = work.tile([128, 128], bf16, tag="rot")
                    nc.vector.tensor_mul(out=rot, in0=ynorm, in1=cos_t)
                    nc.vector.tensor_mul(out=shifted, in0=shifted, in1=sin_t)
                    nc.vector.tensor_add(out=rot, in0=rot, in1=shifted)

                    ptr = psT.tile([128, 128], bf16, tag="tr")
                    nc.tensor.transpose(ptr, rot, ident)
                    nc.any.tensor_copy(out=dstT[:, tcol:tcol + 128], in_=ptr)

                def finish_tile(pq, pkv, i):
                    si = i % NTB
                    for h in range(HL):
                        norm_rope_transpose(
                            pq[:, h * D:(h + 1) * D],
                            tabs["cosq"][:, si, :], tabs["sinq"][:, si, :],
                            QT[h], i * 128)
                    norm_rope_transpose(
                        pkv[:, 0:D],
                        tabs["cosk"][:, si, :], tabs["sink"][:, si, :],
                        KT, i * 128)
                    nc.any.tensor_copy(out=VA[:, i, :], in_=pkv[:, D:2 * D])

                pend = None
                for i in range(NT):
                    if i == 0:
                        halves = hst_early
                    else:
                        halves = []
                        for u in range(2):
                            hh = hstp.tile([128, HC, 128], bf16, name="hsth",
                                           tag="h")
                            nc.sync.dma_start(out=hh, in_=hsT_h[:, i, u, :, :])
                            halves.append(hh)
                    pq = psQ.tile([128, QF], fp32, tag="Q")
                    pkv = psKV.tile([128, 2 * D], fp32, tag="KV")
                    for c in range(NC):
                        hst_c = halves[c // HC][:, c % HC, :]
                        nc.tensor.matmul(pq, hst_c, wq_sb[:, c, :],
                                         start=(c == 0), stop=(c == NC - 1))
                        nc.tensor.matmul(pkv, hst_c, wkv_sb[:, c, :],
                                         start=(c == 0), stop=(c == NC - 1))
                    if pend is not None:
                        finish_tile(*pend)
                    pend = (pq, pkv, i)
                finish_tile(*pend)

            # ============ phase 2: attention with interleaved o_proj ============
            with (
                tc.tile_pool(name="psS", bufs=2, space="PSUM") as psS,
                tc.tile_pool(name="psV", bufs=2, space="PSUM") as psV,
                tc.tile_pool(name="psO", bufs=2, space="PSUM") as psO,
            ):
                fill = []      # pending o_proj emitters (always-ready PE work)
                copy_flip = [0]

                def drain_fill(n):
                    for _ in range(min(n, len(fill))):
                        fill.pop(0)()

                def make_oproj_group(ot_blk, b, j, it, n):
                    def emit():
                        po = psO.tile([128, 512], fp32, tag="O")
                        for h in range(HL):
                            nc.tensor.matmul(
                                po,
                                ot_blk[:, h, it * 128:(it + 1) * 128],
                                wo_sb[:, h, n * 512:(n + 1) * 512],
                                start=(h == 0), stop=(h == HL - 1))
                        ost = ostage.tile([128, 512], bf16, tag="ost")
                        copy_flip[0] ^= 1
                        if copy_flip[0]:
                            nc.scalar.copy(out=ost, in_=po)
                        else:
                            nc.vector.tensor_copy(out=ost, in_=po)
                        t0 = b * S + j * 512 + it * 128
                        nc.sync.dma_start(
                            out=out[t0:t0 + 128, n * 512:(n + 1) * 512], in_=ost)
                    return emit

                for b in range(B):
                    t0 = b * S
                    k0 = b * NTB
                    for j in range(NJ):
                        qcol = t0 + j * 512
                        K = 4 * (j + 1)
                        NP = K // 2
                        ot_blk = otbp.tile([128, HL, 512], bf16, name="otb")
                        for h in range(HL):
                            prs = prsp.tile([128, 2, 512], bf16, tag="prsum")
                            opvT = psV.tile([128, 512], fp32, tag="V")
                            # pair 0's exp writes straight into the prs
                            # accumulator; later pairs are added into it only
                            # AFTER their (lagged) PV matmuls have consumed
                            # them, so in-place accumulation never races PV.
                            pvq = []   # pending (pair_idx, prpair) lagged PV+add
                            def flush_pv(pp, prpair):
                                for half in range(2):
                                    kk = 2 * pp + half
                                    nc.tensor.matmul(
                                        opvT, VA[:, k0 + kk, :],
                                        prpair[:, half, :],
                                        start=(kk == 0), stop=(kk == K - 1))
                                if pp > 0:
                                    nc.vector.tensor_add(
                                        out=prs, in0=prs, in1=prpair)
                            for p in range(NP):
                                spair = psS.tile([128, 2, 512], fp32, tag="S")
                                for half in range(2):
                                    k = 2 * p + half
                                    nc.tensor.matmul(
                                        spair[:, half, :],
                                        KT[:, t0 + k * 128: t0 + (k + 1) * 128],
                                        QT[h][:, qcol:qcol + 512],
                                        start=True, stop=True)
                                if p == 0:
                                    prpair = prs
                                else:
                                    prpair = probsp.tile(
                                        [128, 2, 512], bf16, tag="pr")
                                nc.scalar.activation(
                                    out=prpair, in_=spair,
                                    func=mybir.ActivationFunctionType.Exp,
                                    scale=SCALE)
                                for half in range(2):
                                    k = 2 * p + half
                                    if k >= 4 * j:
                                        nc.vector.tensor_mul(
                                            out=prpair[:, half, :],
                                            in0=prpair[:, half, :],
                                            in1=mask_slice(k - 4 * j))
                                pvq.append((p, prpair))
                                drain_fill(2)
                                if len(pvq) > 1:
                                    flush_pv(*pvq.pop(0))
                            for pp, prpair in pvq:
                                flush_pv(pp, prpair)
                            # denominator: fold pair halves in place, ones^T @
                            # prs[:,0] -> [1,512], fast reciprocal, broadcast,
                            # normalize during the opvT PSUM->SBUF copy.
                            nc.vector.tensor_add(
                                out=prs[:, 0, :], in0=prs[:, 0, :],
                                in1=prs[:, 1, :])
                            dn = psV.tile([1, 512], fp32, tag="V")
                            nc.tensor.matmul(dn, ones_col, prs[:, 0, :],
                                             start=True, stop=True)
                            rbc = rbcp.tile([128, 512], fp32, tag="rbc")
                            nc.vector.reciprocal_approx_fast(
                                out=rbc[0:1, :], in_=dn)
                            nc.gpsimd.partition_broadcast(rbc, rbc[0:1, :])
                            nc.vector.tensor_mul(
                                out=ot_blk[:, h, :], in0=opvT, in1=rbc)
                        fill.extend(
                            make_oproj_group(ot_blk, b, j, it, n)
                            for it in range(4) for n in range(NO))
                drain_fill(len(fill))

    nc.finalize()
    return nc


def _get_nc():
    if "nc" not in _NC_CACHE:
        _NC_CACHE["nc"] = _build()
    return _NC_CACHE["nc"]


def _host_prep(hidden_states, wq, wk, wv, wo, q_norm_w, k_norm_w, position_ids):
    bf = ml_dtypes.bfloat16
    NT, NTB, NC_, HL_ = T // 128, S // 128, HID // 128, HL
    hs = np.asarray(hidden_states, dtype=np.float32).reshape(T, HID)
    # [128(p), NT, NC, 128(t)]: partition-major so DMA lines are contiguous
    hsT = np.ascontiguousarray(
        hs.reshape(NT, 128, NC_, 128).transpose(3, 0, 2, 1)).astype(bf)

    # RoPE tables with norm weights folded in (positions are identical
    # across batches for this problem's arange position_ids).
    pos = np.asarray(position_ids)[0].astype(np.float64)
    inv_freq = 1.0 / (THETA ** (np.arange(0, D, 2, dtype=np.float64) / D))
    ang = pos[:, None] * inv_freq
    emb = np.concatenate([ang, ang], axis=-1)
    cos = np.cos(emb).astype(np.float32)
    sin = np.sin(emb).astype(np.float32)

    def fold(w):
        w = np.asarray(w, dtype=np.float32)
        w_shift = np.concatenate([w[D // 2:], w[:D // 2]])
        sgn = np.concatenate([-np.ones(D // 2, np.float32), np.ones(D // 2, np.float32)])
        return (cos * w).astype(bf), (sin * w_shift * sgn).astype(bf)

    def ptile(a, nblk):
        # [nblk*128, F] -> [128, nblk, F] partition-major
        a = np.ascontiguousarray(a)
        return np.ascontiguousarray(
            a.reshape(nblk, 128, a.shape[-1]).transpose(1, 0, 2))

    cq, sq_ = fold(q_norm_w)
    ck, sk_ = fold(k_norm_w)
    cq, sq_, ck, sk_ = (ptile(x, NTB) for x in (cq, sq_, ck, sk_))

    wq = np.asarray(wq, dtype=np.float32)
    wk = np.asarray(wk, dtype=np.float32)
    wv = np.asarray(wv, dtype=np.float32)
    wo = np.asarray(wo, dtype=np.float32)

    in_maps = []
    for c in range(NCORES):
        qs = slice(c * QF, (c + 1) * QF)
        ks = slice(c * D, (c + 1) * D)
        in_maps.append({
            "hsT": hsT,
            "wq": ptile(wq[:, qs].astype(bf), NC_),
            "wkv": ptile(np.concatenate(
                [wk[:, ks], wv[:, ks]], axis=1).astype(bf), NC_),
            "wo": ptile(wo[qs, :].astype(bf), HL_),
            "cosq": cq, "sinq": sq_, "cosk": ck, "sink": sk_,
        })
    return in_maps


def kernel(hidden_states, wq, wk, wv, wo, q_norm_w, k_norm_w, position_ids,
           _trace=False):
    from concourse.bass_utils import run_bass_kernel_spmd

    nc = _get_nc()
    in_maps = _host_prep(hidden_states, wq, wk, wv, wo,
                         q_norm_w, k_norm_w, position_ids)
    res = run_bass_kernel_spmd(nc, in_maps, core_ids=list(range(NCORES)),
                               trace=_trace)
    total = np.zeros((T, HID), dtype=np.float32)
    for r in res.results:
        total += r["out"]
    out = total.reshape(B, S, HID)
    if _trace:
        return out, res
    return out
